# revision 1
# baseline (speedup 1.0000x reference)
"""Trainium2 Bass kernel for the retrieval-KNN correlation problem.

Problem (per batch element b):
    idx[k,p]   = x[b,k,p] + 64*y[b,k,p]              (pixel coords into ref map)
    S[k,p]     = sum_c ref[b,c,idx[k,p]] * inp[b,c,p]
    best[p]    = argmax_k S[k,p]        (first occurrence on ties)
    out_x[p]   = x[b,best[p],p],  out_y[p] = y[b,best[p],p]

Sharding: 8 cores = (batch b = core//2, pixel half = core%2). Each core owns
all 16 candidates for 2048 contiguous pixels of one batch element, so there is
no cross-core communication.

Per-core dataflow:
  - ref[b] resident in SBUF packed as (128, 4*4096): partition p holds the
    channel quad 4*(p%64)+j interleaved per pixel, duplicated across the two
    partition halves. One GPSIMD ap_gather with d=4 then serves TWO
    candidates at once (each 16-partition group consumes its own index list,
    so partitions 0-63 gather candidate ka and 64-127 gather kb). The Q7
    gather cost is ~26ns/index + ~1.25ns/element (HW-measured), so quartering
    the per-candidate index count cuts the gather wall (the dominant cost).
  - ap_gather indices are shared across partitions, wrapped per 16-partition
    group; built on-chip from x/y via strided PE transposes (a strided DMA
    load of the same layout costs ~4096 64B descriptors at ~600ns each).
  - DVE de-interleaves with two strided multiplies against the same-packed
    inp tile; PE reduces over channels with one-hot-column fp32 matmuls,
    both channel parities accumulating into one (16,2048) PSUM tile.
  - PE transposes S to pixel-major (128,16) tiles; DVE computes the
    first-occurrence argmax via a reverse-weight trick and selects x/y.

HW-verified: exact match vs the jax reference (rel err 0.0), 620 us.
"""

import numpy as np
from contextlib import ExitStack

import concourse.bacc as bacc
import concourse.bass as bass
import concourse.mybir as mybir
import concourse.tile as tile
from concourse import bass_utils

B, K, CN, H, W = 4, 16, 256, 64, 64
HW = H * W            # 4096 pixels per batch element
HALF = HW // 2        # 2048 pixels per core
NCORES = 8
NT = HALF // 128      # 16 pixel tiles of 128

f32 = mybir.dt.float32
f32r = mybir.dt.float32r
i16 = mybir.dt.int16


def build_program():
    nc = bacc.Bacc("TRN2", target_bir_lowering=False, debug=False)

    ref_d = nc.dram_tensor("ref", (128, 4 * HW), f32, kind="ExternalInput")
    inp_d = nc.dram_tensor("inp", (128, 4 * HALF), f32, kind="ExternalInput")
    xh_d = nc.dram_tensor("xh", (K, HALF), f32, kind="ExternalInput")
    yh_d = nc.dram_tensor("yh", (K, HALF), f32, kind="ExternalInput")
    ident_d = nc.dram_tensor("ident", (128, 128), f32, kind="ExternalInput")
    eye16_d = nc.dram_tensor("eye16", (128, 16 * K), f32, kind="ExternalInput")
    revc_d = nc.dram_tensor("revc", (128, 16 * NT), f32, kind="ExternalInput")
    ox_d = nc.dram_tensor("ox", (128, NT), f32, kind="ExternalOutput")
    oy_d = nc.dram_tensor("oy", (128, NT), f32, kind="ExternalOutput")

    with ExitStack() as ctx:
        tc = ctx.enter_context(tile.TileContext(nc))
        pers = ctx.enter_context(tc.tile_pool(name="pers", bufs=1))
        gpool = ctx.enter_context(tc.tile_pool(name="g", bufs=3))
        tpool = ctx.enter_context(tc.tile_pool(name="t", bufs=3))
        ps_s = ctx.enter_context(tc.tile_pool(name="ps_s", bufs=1, space="PSUM"))
        ps_tp = ctx.enter_context(tc.tile_pool(name="ps_tp", bufs=2, space="PSUM"))

        # ---- persistent tiles -------------------------------------------------
        rt = pers.tile([128, 4 * HW], f32, tag="refp")
        it = pers.tile([128, 4 * HALF], f32, tag="inpp")
        xn = pers.tile([K, HALF], f32, tag="xn")
        yn = pers.tile([K, HALF], f32, tag="yn")
        ri = pers.tile([128, 16 * K], f32, tag="ri")     # idx in R-layout, fp32
        ident = pers.tile([128, 128], f32, tag="ident")
        eye16 = pers.tile([128, 16 * K], f32, tag="eye16")
        revc = pers.tile([128, 16 * NT], f32, tag="revc")
        wk = [pers.tile([128, 64], i16, tag=f"w{k}", name=f"w{k}") for k in range(K)]
        xt = pers.tile([128, 16 * NT], f32, tag="xt")    # x, pixel-major
        yt = pers.tile([128, 16 * NT], f32, tag="yt")
        st = pers.tile([128, 16 * NT], f32, tag="st")    # S^T, pixel-major
        ssb = pers.tile([K, HALF], f32, tag="ssb")

        # ---- loads ------------------------------------------------------------
        nc.sync.dma_start(ident[:], ident_d.ap())
        nc.sync.dma_start(eye16[:], eye16_d.ap())
        nc.sync.dma_start(revc[:], revc_d.ap())
        nc.sync.dma_start(rt[:], ref_d.ap())
        nc.sync.dma_start(it[:], inp_d.ap())
        nc.sync.dma_start(xn[:], xh_d.ap())
        nc.sync.dma_start(yn[:], yh_d.ap())

        ident16 = ident[0:16, 0:16]

        # ---- index pipeline: idx = x + 64*y in R-layout -----------------------
        # ri[r, 16k+c] = idx[k, 16r+c], built via strided PE transposes of the
        # contiguous xn/yn loads (a strided DMA here would cost ~4096 64B
        # descriptors at ~600ns each).
        xn_g = xn[:].rearrange("k (r c) -> k r c", c=16)
        yn_g = yn[:].rearrange("k (r c) -> k r c", c=16)
        ri_g = ri[:].rearrange("p (k c) -> p k c", c=16)
        for c16 in range(16):
            txp = ps_tp.tile([128, K], f32, tag="tp", name=f"txp{c16}")
            nc.tensor.transpose(txp[:], xn_g[:, :, c16], ident16)
            typ = ps_tp.tile([128, K], f32, tag="tp2", name=f"typ{c16}")
            nc.tensor.transpose(typ[:], yn_g[:, :, c16], ident16)
            rtmp = tpool.tile([128, K], f32, tag="rtmp", name=f"rtmp{c16}")
            nc.vector.tensor_scalar_mul(rtmp[:], typ[:], 64.0)
            nc.vector.tensor_add(ri_g[:, :, c16], rtmp[:], txp[:])
        for pr in range(K // 2):
            ka, kb = 2 * pr, 2 * pr + 1
            rep = tpool.tile([128, 128], f32, tag="rep", name=f"rep{pr}")
            ra = ri[:, 16 * ka:16 * (ka + 1)].unsqueeze(1).broadcast_to((128, 4, 16))
            rb = ri[:, 16 * kb:16 * (kb + 1)].unsqueeze(1).broadcast_to((128, 4, 16))
            nc.vector.tensor_copy(rep[:, 0:64].rearrange("p (g c) -> p g c", c=16), ra)
            nc.vector.tensor_copy(rep[:, 64:128].rearrange("p (g c) -> p g c", c=16), rb)
            for h in range(2):
                wp = ps_tp.tile([128, 64], f32, tag="tp", name=f"wp{pr}_{h}")
                nc.tensor.transpose(wp[:], rep[64 * h:64 * (h + 1), :],
                                    ident[64 * h:64 * (h + 1), 64 * h:64 * (h + 1)])
                nc.scalar.copy(wk[2 * pr + h][:], wp[:])

        # ---- x/y to pixel-major (128,16) tiles via PE transpose ---------------
        for t in range(NT):
            xp = ps_tp.tile([128, 16], f32, tag="tp")
            nc.tensor.transpose(xp[:], xn[:, 128 * t:128 * (t + 1)], ident16)
            nc.scalar.copy(xt[:, 16 * t:16 * (t + 1)], xp[:])
            yp = ps_tp.tile([128, 16], f32, tag="tp")
            nc.tensor.transpose(yp[:], yn[:, 128 * t:128 * (t + 1)], ident16)
            nc.scalar.copy(yt[:, 16 * t:16 * (t + 1)], yp[:])

        # ---- main loop: gather, multiply, reduce over channels ----------------
        PH = HALF // 2   # 1024 pixels per gather instruction
        s_ps = ps_s.tile([K, HALF], f32, tag="s")
        for pr in range(K // 2):
            for h in range(2):
                g = gpool.tile([128, 4 * PH], f32, tag="g", bufs=2)
                nc.gpsimd.ap_gather(
                    g[:].rearrange("p (i d) -> p i d", d=4),
                    rt[:].rearrange("p (e d) -> p e d", d=4),
                    wk[2 * pr + h][:],
                    channels=128, num_elems=HW, d=4, num_idxs=PH,
                )
                tt = tpool.tile([128, 4 * PH], f32, tag="t", bufs=2)
                gv = g[:].rearrange("p (i d) -> p i d", d=4)
                iv = it[:].rearrange("p (hh i d) -> p hh i d", hh=2, d=4)
                for par in range(4):
                    nc.vector.tensor_mul(tt[:, par * PH:(par + 1) * PH],
                                         gv[:, :, par], iv[:, h, :, par])
                for par in range(4):
                    for q in range(PH // 512):
                        nc.tensor.matmul(
                            s_ps[:, PH * h + 512 * q:PH * h + 512 * (q + 1)],
                            lhsT=eye16[:, 16 * pr:16 * (pr + 1)],
                            rhs=tt[:, par * PH + 512 * q:par * PH + 512 * (q + 1)],
                            start=(pr == 0 and par == 0),
                            stop=(pr == K // 2 - 1 and par == 3),
                        )

        nc.scalar.copy(ssb[:], s_ps[:])
        for t in range(NT):
            stp = ps_tp.tile([128, 16], f32, tag="tp")
            nc.tensor.transpose(stp[:], ssb[:, 128 * t:128 * (t + 1)], ident16)
            nc.scalar.copy(st[:, 16 * t:16 * (t + 1)], stp[:])

        # ---- argmax (first occurrence) + offset select ------------------------
        def grp(ap):  # (128, 256) -> (128, 16, 16)
            return ap.rearrange("p (t j) -> p t j", j=16)

        gmax = pers.tile([128, NT], f32, tag="gmax")
        ohall = pers.tile([128, 16 * NT], f32, tag="ohall")
        t1 = pers.tile([128, 16 * NT], f32, tag="t1")
        r1 = pers.tile([128, NT], f32, tag="r1")
        oh1 = pers.tile([128, 16 * NT], f32, tag="oh1")
        sel = pers.tile([128, 16 * NT], f32, tag="sel")
        oxv = pers.tile([128, NT], f32, tag="oxv")
        oyv = pers.tile([128, NT], f32, tag="oyv")

        nc.vector.tensor_reduce(gmax[:], grp(st[:]), axis=mybir.AxisListType.X,
                                op=mybir.AluOpType.max)
        gb = gmax[:].unsqueeze(2).broadcast_to((128, NT, 16))
        nc.vector.tensor_tensor(grp(ohall[:]), grp(st[:]), gb,
                                op=mybir.AluOpType.is_equal)
        nc.vector.tensor_mul(t1[:], ohall[:], revc[:])
        nc.vector.tensor_reduce(r1[:], grp(t1[:]), axis=mybir.AxisListType.X,
                                op=mybir.AluOpType.max)
        rb = r1[:].unsqueeze(2).broadcast_to((128, NT, 16))
        nc.vector.tensor_tensor(grp(oh1[:]), grp(t1[:]), rb,
                                op=mybir.AluOpType.is_equal)
        nc.vector.tensor_mul(sel[:], oh1[:], xt[:])
        nc.vector.tensor_reduce(oxv[:], grp(sel[:]), axis=mybir.AxisListType.X,
                                op=mybir.AluOpType.add)
        nc.vector.tensor_mul(sel[:], oh1[:], yt[:])
        nc.vector.tensor_reduce(oyv[:], grp(sel[:]), axis=mybir.AxisListType.X,
                                op=mybir.AluOpType.add)

        nc.sync.dma_start(ox_d.ap(), oxv[:])
        nc.sync.dma_start(oy_d.ap(), oyv[:])

    nc.compile()
    return nc


def make_consts():
    ident = np.eye(128, dtype=np.float32)
    # EE[p, 16*pr + m] = 1 iff (m == 2*pr and p < 64) or (m == 2*pr+1 and p >= 64)
    eye16 = np.zeros((128, 256), dtype=np.float32)
    for pr in range(8):
        eye16[:64, 16 * pr + 2 * pr] = 1.0
        eye16[64:, 16 * pr + 2 * pr + 1] = 1.0
    revc = np.tile(
        np.tile((16.0 - np.arange(16, dtype=np.float32)), NT).reshape(1, 16 * NT),
        (128, 1),
    )
    return ident, eye16, np.ascontiguousarray(revc)


def pack_quads(a):
    # (256, n) -> (128, 4n): partition p holds channels 4*(p%64)+j interleaved,
    # duplicated across the two partition halves (each half sees all channels).
    cn, n = a.shape
    half = a.reshape(cn // 4, 4, n).transpose(0, 2, 1).reshape(cn // 4, 4 * n)
    return np.ascontiguousarray(np.tile(half, (2, 1)))


def make_in_maps(input_features, ref_features, aggregated_x, aggregated_y):
    ident, eye16, revc = make_consts()
    in_maps = []
    for core in range(NCORES):
        b, h = core // 2, core % 2
        sl = slice(h * HALF, (h + 1) * HALF)
        in_maps.append({
            "ref": pack_quads(ref_features[b].reshape(CN, HW)),
            "inp": pack_quads(input_features[b].reshape(CN, HW)[:, sl]),
            "xh": np.ascontiguousarray(aggregated_x[b].reshape(K, HW)[:, sl]),
            "yh": np.ascontiguousarray(aggregated_y[b].reshape(K, HW)[:, sl]),
            "ident": ident,
            "eye16": eye16,
            "revc": revc,
        })
    return in_maps


def assemble_outputs(results):
    offset_x = np.empty((B, 1, H, W), dtype=np.float32)
    offset_y = np.empty((B, 1, H, W), dtype=np.float32)
    for core in range(NCORES):
        b, h = core // 2, core % 2
        sl = slice(h * HALF, (h + 1) * HALF)
        # ox[p, t] holds pixel t*128+p -> transpose to pixel order
        offset_x[b, 0].reshape(HW)[sl] = results[core]["ox"].T.reshape(HALF)
        offset_y[b, 0].reshape(HW)[sl] = results[core]["oy"].T.reshape(HALF)
    return offset_x, offset_y


_PROGRAM = None


def kernel(input_features, ref_features, aggregated_x, aggregated_y):
    global _PROGRAM
    if _PROGRAM is None:
        _PROGRAM = build_program()
    nc = _PROGRAM
    in_maps = make_in_maps(input_features, ref_features, aggregated_x, aggregated_y)
    res = bass_utils.run_bass_kernel_spmd(nc, in_maps, core_ids=list(range(NCORES)))
    return assemble_outputs(res.results)



# revision 6
# speedup vs baseline: 1.6391x; 1.6391x over previous
"""Trainium2 Bass kernel for the retrieval-KNN correlation problem.

Problem (per batch element b):
    idx[k,p]   = x[b,k,p] + 64*y[b,k,p]              (pixel coords into ref map)
    S[k,p]     = sum_c ref[b,c,idx[k,p]] * inp[b,c,p]
    best[p]    = argmax_k S[k,p]        (first occurrence on ties)
    out_x[p]   = x[b,best[p],p],  out_y[p] = y[b,best[p],p]

Sharding: 8 cores = (batch b = core//2, pixel half = core%2). Each core owns
all 16 candidates for 2048 contiguous pixels of one batch element, so there is
no cross-core communication.

Per-core dataflow (DMA-gather version):
  - ref[b] stays in DRAM, stored pixel-major (4096 rows x 256 channels, 1KB
    rows). The gather runs as SWDGE dma_gather: each int16 index fetches one
    contiguous 1KB row straight from HBM into SBUF (dst[i%128, i//128, :]).
    Two calls per candidate (1024 indices / 1MB each -- the SWDGE queue ring
    holds at most 1024 descriptors, HW-verified cliff); a call's descriptors
    spread across all 16 DMA engines, so each call runs at the ~360GB/s
    aggregate DMA roofline (~2.9us) and the 32 calls stream back-to-back
    (~93us total for 33.5MB). Descriptor generation on GPSIMD is cheap
    (994ns + 0.34ns/desc per call) and overlaps the previous transfer.
    This replaces the previous GPSIMD ap_gather ucode (~26ns/index serial on
    the Q7 cores, ~535us busy) -- the gather is now memory-bound.
  - Indices (x + 64*y as int16, wrapped in 16 partitions per the SWDGE index
    layout) and the pixel-major transposes of inp/x/y are precomputed on the
    host in make_in_maps, so the kernel has no on-chip index pipeline and no
    PE/PSUM use at all.
  - DVE consumes each gathered candidate as it lands: in-place multiply
    against the resident pixel-major inp tile, then a segmented 256->1
    add-reduce writes S directly in pixel-major order (stride-16 columns of
    st). ~3.7us per candidate, fully hidden under the next gather.
  - Final first-occurrence argmax via the reverse-weight trick + x/y select,
    all on DVE in pixel-major layout (no transposes needed).

HW-verified: exact match vs the jax reference (rel err 0.0).
"""

import numpy as np
from contextlib import ExitStack

import concourse.bacc as bacc
import concourse.bass as bass
import concourse.mybir as mybir
import concourse.tile as tile
from concourse import bass_utils

B, K, CN, H, W = 4, 16, 256, 64, 64
HW = H * W            # 4096 pixels per batch element
HALF = HW // 2        # 2048 pixels per core
NCORES = 8
NT = HALF // 128      # 16 pixel tiles of 128
NIW = HALF // 16      # 128 wrapped-index slots per candidate

f32 = mybir.dt.float32
i16 = mybir.dt.int16


def build_program():
    nc = bacc.Bacc("TRN2", target_bir_lowering=False, debug=False)

    refT_d = nc.dram_tensor("refT", (HW, CN), f32, kind="ExternalInput")
    it_d = nc.dram_tensor("it", (128, NT * CN), f32, kind="ExternalInput")
    wi_d = nc.dram_tensor("wi", (128, K * NIW), i16, kind="ExternalInput")
    xt_d = nc.dram_tensor("xt", (128, NT * K), f32, kind="ExternalInput")
    yt_d = nc.dram_tensor("yt", (128, NT * K), f32, kind="ExternalInput")
    revc_d = nc.dram_tensor("revc", (128, NT * K), f32, kind="ExternalInput")
    ox_d = nc.dram_tensor("ox", (128, NT), f32, kind="ExternalOutput")
    oy_d = nc.dram_tensor("oy", (128, NT), f32, kind="ExternalOutput")

    with ExitStack() as ctx:
        tc = ctx.enter_context(tile.TileContext(nc))
        pers = ctx.enter_context(tc.tile_pool(name="pers", bufs=1))
        gpool = ctx.enter_context(tc.tile_pool(name="g", bufs=4))

        # ---- persistent tiles -------------------------------------------------
        it = pers.tile([128, NT * CN], f32, tag="it")    # inp, pixel-major
        wi = pers.tile([128, K * NIW], i16, tag="wi")
        xt = pers.tile([128, NT * K], f32, tag="xt")     # x, pixel-major
        yt = pers.tile([128, NT * K], f32, tag="yt")
        revc = pers.tile([128, NT * K], f32, tag="revc")
        st = pers.tile([128, NT * K], f32, tag="st")     # S, pixel-major

        st_g = st[:].rearrange("p (t j) -> p t j", j=K)

        # wi first (the first gather's descriptor generation reads it); the
        # remaining loads overlap the first gather's transfer.
        nc.sync.dma_start(wi[:], wi_d.ap())

        CH = 1024                 # indices per dma_gather (SWDGE ring limit)
        JW = CH // 128            # 8 pixel tiles per chunk
        for k in range(K):
            for h2 in range(HALF // CH):
                g = gpool.tile([128, JW * CN], f32, tag="g", name=f"g{k}_{h2}")
                nc.gpsimd.dma_gather(
                    g[:].rearrange("p (j e) -> p j e", e=CN),
                    refT_d[:],
                    wi[:, k * NIW + (CH // 16) * h2:
                        k * NIW + (CH // 16) * (h2 + 1)],
                    CH, CH, CN,
                    queue_num=0,
                )
                if k == 0 and h2 == 0:
                    nc.sync.dma_start(it[:], it_d.ap())
                    nc.sync.dma_start(xt[:], xt_d.ap())
                    nc.sync.dma_start(yt[:], yt_d.ap())
                    nc.sync.dma_start(revc[:], revc_d.ap())
                nc.vector.tensor_mul(g[:], g[:], it[:, JW * CN * h2:
                                                    JW * CN * (h2 + 1)])
                nc.vector.tensor_reduce(
                    st_g[:, JW * h2:JW * (h2 + 1), k],
                    g[:].rearrange("p (j e) -> p j e", e=CN),
                    axis=mybir.AxisListType.X, op=mybir.AluOpType.add,
                )

        # ---- argmax (first occurrence) + offset select ------------------------
        def grp(ap):  # (128, 256) -> (128, 16, 16)
            return ap.rearrange("p (t j) -> p t j", j=K)

        gmax = pers.tile([128, NT], f32, tag="gmax")
        t1 = pers.tile([128, NT * K], f32, tag="t1")
        r1 = pers.tile([128, NT], f32, tag="r1")
        oh1 = pers.tile([128, NT * K], f32, tag="oh1")
        sel = pers.tile([128, NT * K], f32, tag="sel")
        oxv = pers.tile([128, NT], f32, tag="oxv")
        oyv = pers.tile([128, NT], f32, tag="oyv")

        nc.vector.tensor_reduce(gmax[:], st_g, axis=mybir.AxisListType.X,
                                op=mybir.AluOpType.max)
        gb = gmax[:].unsqueeze(2).broadcast_to((128, NT, K))
        nc.vector.tensor_tensor(grp(t1[:]), st_g, gb,
                                op=mybir.AluOpType.is_equal)
        nc.vector.tensor_mul(t1[:], t1[:], revc[:])
        nc.vector.tensor_reduce(r1[:], grp(t1[:]), axis=mybir.AxisListType.X,
                                op=mybir.AluOpType.max)
        rb = r1[:].unsqueeze(2).broadcast_to((128, NT, K))
        nc.vector.tensor_tensor(grp(oh1[:]), grp(t1[:]), rb,
                                op=mybir.AluOpType.is_equal)
        nc.vector.tensor_mul(sel[:], oh1[:], xt[:])
        nc.vector.tensor_reduce(oxv[:], grp(sel[:]), axis=mybir.AxisListType.X,
                                op=mybir.AluOpType.add)
        nc.vector.tensor_mul(sel[:], oh1[:], yt[:])
        nc.vector.tensor_reduce(oyv[:], grp(sel[:]), axis=mybir.AxisListType.X,
                                op=mybir.AluOpType.add)

        nc.sync.dma_start(ox_d.ap(), oxv[:])
        nc.sync.dma_start(oy_d.ap(), oyv[:])

    nc.compile()
    return nc


def make_consts():
    # revc[p, t*16 + j] = 16 - j : reverse weight for first-occurrence argmax
    return np.tile(
        np.tile((16.0 - np.arange(16, dtype=np.float32)), NT).reshape(1, NT * K),
        (128, 1),
    ).copy()


def make_in_maps(input_features, ref_features, aggregated_x, aggregated_y):
    revc = make_consts()
    in_maps = []
    for core in range(NCORES):
        b, h = core // 2, core % 2
        sl = slice(h * HALF, (h + 1) * HALF)
        inp = np.asarray(input_features[b]).reshape(CN, HW)[:, sl]
        xh = np.asarray(aggregated_x[b]).reshape(K, HW)[:, sl]
        yh = np.asarray(aggregated_y[b]).reshape(K, HW)[:, sl]
        idx = (xh + 64.0 * yh).astype(np.int16)              # (K, HALF)
        # SWDGE wrapped index layout: list elem i at partition i%16, slot
        # i//16, replicated across the 8 16-partition blocks.
        wi = np.tile(
            idx.reshape(K, NIW, 16).transpose(2, 0, 1).reshape(16, -1),
            (8, 1),
        )
        in_maps.append({
            "refT": np.ascontiguousarray(
                np.asarray(ref_features[b]).reshape(CN, HW).T),
            "it": np.ascontiguousarray(
                inp.reshape(CN, NT, 128).transpose(2, 1, 0).reshape(128, -1)),
            "wi": np.ascontiguousarray(wi),
            "xt": np.ascontiguousarray(
                xh.reshape(K, NT, 128).transpose(2, 1, 0).reshape(128, -1)),
            "yt": np.ascontiguousarray(
                yh.reshape(K, NT, 128).transpose(2, 1, 0).reshape(128, -1)),
            "revc": revc,
        })
    return in_maps


def assemble_outputs(results):
    offset_x = np.empty((B, 1, H, W), dtype=np.float32)
    offset_y = np.empty((B, 1, H, W), dtype=np.float32)
    for core in range(NCORES):
        b, h = core // 2, core % 2
        sl = slice(h * HALF, (h + 1) * HALF)
        # ox[p, t] holds pixel t*128+p -> transpose to pixel order
        offset_x[b, 0].reshape(HW)[sl] = results[core]["ox"].T.reshape(HALF)
        offset_y[b, 0].reshape(HW)[sl] = results[core]["oy"].T.reshape(HALF)
    return offset_x, offset_y


_PROGRAM = None


def kernel(input_features, ref_features, aggregated_x, aggregated_y):
    global _PROGRAM
    if _PROGRAM is None:
        _PROGRAM = build_program()
    nc = _PROGRAM
    in_maps = make_in_maps(input_features, ref_features, aggregated_x, aggregated_y)
    res = bass_utils.run_bass_kernel_spmd(nc, in_maps, core_ids=list(range(NCORES)))
    return assemble_outputs(res.results)


# revision 8
# speedup vs baseline: 3.2670x; 1.9932x over previous
"""Trainium2 Bass kernel for the retrieval-KNN correlation problem.

Problem (per batch element b):
    idx[k,p]   = x[b,k,p] + 64*y[b,k,p]              (pixel coords into ref map)
    S[k,p]     = sum_c ref[b,c,idx[k,p]] * inp[b,c,p]
    best[p]    = argmax_k S[k,p]        (first occurrence on ties)
    out_x[p]   = x[b,best[p],p],  out_y[p] = y[b,best[p],p]

Sharding: 8 cores = (batch b = core//2, pixel half = core%2). Each core owns
all 16 candidates for 2048 contiguous pixels of one batch element, so there is
no cross-core communication.

Per-core dataflow (DMA-gather version):
  - ref[b] stays in DRAM, stored pixel-major (4096 rows x 256 channels, 1KB
    rows). The gather runs as SWDGE dma_gather: each int16 index fetches one
    contiguous 1KB row straight from HBM into SBUF (dst[i%128, i//128, :]).
    Two calls per candidate (1024 indices / 1MB each -- the SWDGE queue ring
    holds at most 1024 descriptors, HW-verified cliff); a call's descriptors
    spread across all 16 DMA engines, so each call runs at the ~360GB/s
    aggregate DMA roofline (~2.9us) and the 32 calls stream back-to-back
    (~93us total for 33.5MB). Descriptor generation on GPSIMD is cheap
    (994ns + 0.34ns/desc per call) and overlaps the previous transfer.
    This replaces the previous GPSIMD ap_gather ucode (~26ns/index serial on
    the Q7 cores, ~535us busy) -- the gather is now memory-bound.
  - Indices (x + 64*y as int16, wrapped in 16 partitions per the SWDGE index
    layout) and the pixel-major transposes of inp/x/y are precomputed on the
    host in make_in_maps, so the kernel has no on-chip index pipeline and no
    PE/PSUM use at all.
  - DVE consumes each gathered candidate as it lands: in-place multiply
    against the resident pixel-major inp tile, then a segmented 256->1
    add-reduce writes S directly in pixel-major order (stride-16 columns of
    st). ~3.7us per candidate, fully hidden under the next gather.
  - Final first-occurrence argmax via the reverse-weight trick + x/y select,
    all on DVE in pixel-major layout (no transposes needed).

HW-verified: exact match vs the jax reference (rel err 0.0).
"""

import numpy as np
from contextlib import ExitStack

import concourse.bacc as bacc
import concourse.bass as bass
import concourse.mybir as mybir
import concourse.tile as tile
from concourse import bass_utils

B, K, CN, H, W = 4, 16, 256, 64, 64
HW = H * W            # 4096 pixels per batch element
HALF = HW // 2        # 2048 pixels per core
NCORES = 8
NT = HALF // 128      # 16 pixel tiles of 128
NIW = HALF // 16      # 128 wrapped-index slots per candidate

f32 = mybir.dt.float32
i16 = mybir.dt.int16


def build_program():
    nc = bacc.Bacc("TRN2", target_bir_lowering=False, debug=False,
                   num_swdge_queues=4)

    refT_d = nc.dram_tensor("refT", (HW, CN), f32, kind="ExternalInput")
    it_d = nc.dram_tensor("it", (128, NT * CN), f32, kind="ExternalInput")
    wi_d = nc.dram_tensor("wi", (128, K * NIW), i16, kind="ExternalInput")
    xt_d = nc.dram_tensor("xt", (128, NT * K), f32, kind="ExternalInput")
    yt_d = nc.dram_tensor("yt", (128, NT * K), f32, kind="ExternalInput")
    revc_d = nc.dram_tensor("revc", (128, NT * K), f32, kind="ExternalInput")
    ox_d = nc.dram_tensor("ox", (128, NT), f32, kind="ExternalOutput")
    oy_d = nc.dram_tensor("oy", (128, NT), f32, kind="ExternalOutput")

    with ExitStack() as ctx:
        tc = ctx.enter_context(tile.TileContext(nc))
        pers = ctx.enter_context(tc.tile_pool(name="pers", bufs=1))
        gpool = ctx.enter_context(tc.tile_pool(name="g", bufs=4))

        # ---- persistent tiles -------------------------------------------------
        it = pers.tile([128, NT * CN], f32, tag="it")    # inp, pixel-major
        wi = pers.tile([128, K * NIW], i16, tag="wi")
        xt = pers.tile([128, NT * K], f32, tag="xt")     # x, pixel-major
        yt = pers.tile([128, NT * K], f32, tag="yt")
        revc = pers.tile([128, NT * K], f32, tag="revc")
        st = pers.tile([128, NT * K], f32, tag="st")     # S, pixel-major

        st_g = st[:].rearrange("p (t j) -> p t j", j=K)

        # wi first (the first gather's descriptor generation reads it); the
        # remaining loads overlap the first gather's transfer.
        nc.sync.dma_start(wi[:], wi_d.ap())

        CH = 1024                 # indices per dma_gather (SWDGE ring limit)
        NCH = HALF // CH          # 2 chunks per candidate
        for k in range(K):
            # Two half-candidate gathers (ring holds max 1024 descriptors)
            # rotated across the 4 SWDGE queues so one queue's ring-drain
            # doesn't stall the next call's descriptor generation.
            g = gpool.tile([128, NT * CN], f32, tag="g", name=f"g{k}")
            for h2 in range(NCH):
                nc.gpsimd.dma_gather(
                    g[:, NT * CN // NCH * h2:NT * CN // NCH * (h2 + 1)]
                        .rearrange("p (j e) -> p j e", e=CN),
                    refT_d[:],
                    wi[:, k * NIW + (CH // 16) * h2:
                        k * NIW + (CH // 16) * (h2 + 1)],
                    CH, CH, CN,
                    queue_num=(k * NCH + h2) % 4,
                )
            if k == 0:
                nc.sync.dma_start(it[:], it_d.ap())
                nc.sync.dma_start(xt[:], xt_d.ap())
                nc.sync.dma_start(yt[:], yt_d.ap())
                nc.sync.dma_start(revc[:], revc_d.ap())
            # Whole-candidate DVE ops (fewer, larger instructions)
            nc.vector.tensor_mul(g[:], g[:], it[:])
            nc.vector.tensor_reduce(
                st_g[:, :, k],
                g[:].rearrange("p (j e) -> p j e", e=CN),
                axis=mybir.AxisListType.X, op=mybir.AluOpType.add,
            )

        # ---- argmax (first occurrence) + offset select ------------------------
        def grp(ap):  # (128, 256) -> (128, 16, 16)
            return ap.rearrange("p (t j) -> p t j", j=K)

        gmax = pers.tile([128, NT], f32, tag="gmax")
        t1 = pers.tile([128, NT * K], f32, tag="t1")
        r1 = pers.tile([128, NT], f32, tag="r1")
        oh1 = pers.tile([128, NT * K], f32, tag="oh1")
        sel = pers.tile([128, NT * K], f32, tag="sel")
        oxv = pers.tile([128, NT], f32, tag="oxv")
        oyv = pers.tile([128, NT], f32, tag="oyv")

        nc.vector.tensor_reduce(gmax[:], st_g, axis=mybir.AxisListType.X,
                                op=mybir.AluOpType.max)
        gb = gmax[:].unsqueeze(2).broadcast_to((128, NT, K))
        nc.vector.tensor_tensor(grp(t1[:]), st_g, gb,
                                op=mybir.AluOpType.is_equal)
        nc.vector.tensor_mul(t1[:], t1[:], revc[:])
        nc.vector.tensor_reduce(r1[:], grp(t1[:]), axis=mybir.AxisListType.X,
                                op=mybir.AluOpType.max)
        rb = r1[:].unsqueeze(2).broadcast_to((128, NT, K))
        nc.vector.tensor_tensor(grp(oh1[:]), grp(t1[:]), rb,
                                op=mybir.AluOpType.is_equal)
        nc.vector.tensor_mul(sel[:], oh1[:], xt[:])
        nc.vector.tensor_reduce(oxv[:], grp(sel[:]), axis=mybir.AxisListType.X,
                                op=mybir.AluOpType.add)
        nc.vector.tensor_mul(sel[:], oh1[:], yt[:])
        nc.vector.tensor_reduce(oyv[:], grp(sel[:]), axis=mybir.AxisListType.X,
                                op=mybir.AluOpType.add)

        nc.sync.dma_start(ox_d.ap(), oxv[:])
        nc.sync.dma_start(oy_d.ap(), oyv[:])

    nc.compile()
    return nc


def make_consts():
    # revc[p, t*16 + j] = 16 - j : reverse weight for first-occurrence argmax
    return np.tile(
        np.tile((16.0 - np.arange(16, dtype=np.float32)), NT).reshape(1, NT * K),
        (128, 1),
    ).copy()


def make_in_maps(input_features, ref_features, aggregated_x, aggregated_y):
    revc = make_consts()
    in_maps = []
    for core in range(NCORES):
        b, h = core // 2, core % 2
        sl = slice(h * HALF, (h + 1) * HALF)
        inp = np.asarray(input_features[b]).reshape(CN, HW)[:, sl]
        xh = np.asarray(aggregated_x[b]).reshape(K, HW)[:, sl]
        yh = np.asarray(aggregated_y[b]).reshape(K, HW)[:, sl]
        idx = (xh + 64.0 * yh).astype(np.int16)              # (K, HALF)
        # SWDGE wrapped index layout: list elem i at partition i%16, slot
        # i//16, replicated across the 8 16-partition blocks.
        wi = np.tile(
            idx.reshape(K, NIW, 16).transpose(2, 0, 1).reshape(16, -1),
            (8, 1),
        )
        in_maps.append({
            "refT": np.ascontiguousarray(
                np.asarray(ref_features[b]).reshape(CN, HW).T),
            "it": np.ascontiguousarray(
                inp.reshape(CN, NT, 128).transpose(2, 1, 0).reshape(128, -1)),
            "wi": np.ascontiguousarray(wi),
            "xt": np.ascontiguousarray(
                xh.reshape(K, NT, 128).transpose(2, 1, 0).reshape(128, -1)),
            "yt": np.ascontiguousarray(
                yh.reshape(K, NT, 128).transpose(2, 1, 0).reshape(128, -1)),
            "revc": revc,
        })
    return in_maps


def assemble_outputs(results):
    offset_x = np.empty((B, 1, H, W), dtype=np.float32)
    offset_y = np.empty((B, 1, H, W), dtype=np.float32)
    for core in range(NCORES):
        b, h = core // 2, core % 2
        sl = slice(h * HALF, (h + 1) * HALF)
        # ox[p, t] holds pixel t*128+p -> transpose to pixel order
        offset_x[b, 0].reshape(HW)[sl] = results[core]["ox"].T.reshape(HALF)
        offset_y[b, 0].reshape(HW)[sl] = results[core]["oy"].T.reshape(HALF)
    return offset_x, offset_y


_PROGRAM = None


def kernel(input_features, ref_features, aggregated_x, aggregated_y):
    global _PROGRAM
    if _PROGRAM is None:
        _PROGRAM = build_program()
    nc = _PROGRAM
    in_maps = make_in_maps(input_features, ref_features, aggregated_x, aggregated_y)
    res = bass_utils.run_bass_kernel_spmd(nc, in_maps, core_ids=list(range(NCORES)))
    return assemble_outputs(res.results)


# revision 10
# speedup vs baseline: 3.3909x; 1.0379x over previous
"""Trainium2 Bass kernel for the retrieval-KNN correlation problem.

Problem (per batch element b):
    idx[k,p]   = x[b,k,p] + 64*y[b,k,p]              (pixel coords into ref map)
    S[k,p]     = sum_c ref[b,c,idx[k,p]] * inp[b,c,p]
    best[p]    = argmax_k S[k,p]        (first occurrence on ties)
    out_x[p]   = x[b,best[p],p],  out_y[p] = y[b,best[p],p]

Sharding: 8 cores = (batch b = core//2, pixel half = core%2). Each core owns
all 16 candidates for 2048 contiguous pixels of one batch element, so there is
no cross-core communication.

Per-core dataflow (DMA-gather version):
  - ref[b] stays in DRAM, stored pixel-major (4096 rows x 256 channels, 1KB
    rows). The gather runs as SWDGE dma_gather: each int16 index fetches one
    contiguous 1KB row straight from HBM into SBUF (dst[i%128, i//128, :]).
    Two calls per candidate (1024 indices / 1MB each -- the SWDGE queue ring
    holds at most 1024 descriptors, HW-verified cliff); a call's descriptors
    spread across all 16 DMA engines, so each call runs at the ~360GB/s
    aggregate DMA roofline (~2.9us) and the 32 calls stream back-to-back
    (~93us total for 33.5MB). Descriptor generation on GPSIMD is cheap
    (994ns + 0.34ns/desc per call) and overlaps the previous transfer.
    This replaces the previous GPSIMD ap_gather ucode (~26ns/index serial on
    the Q7 cores, ~535us busy) -- the gather is now memory-bound.
  - Indices (x + 64*y as int16, wrapped in 16 partitions per the SWDGE index
    layout) and the pixel-major transposes of inp/x/y are precomputed on the
    host in make_in_maps, so the kernel has no on-chip index pipeline and no
    PE/PSUM use at all.
  - DVE consumes each gathered candidate as it lands: in-place multiply
    against the resident pixel-major inp tile, then a segmented 256->1
    add-reduce writes S directly in pixel-major order (stride-16 columns of
    st). ~3.7us per candidate, fully hidden under the next gather.
  - Final first-occurrence argmax via the reverse-weight trick + x/y select,
    all on DVE in pixel-major layout (no transposes needed).

HW-verified: exact match vs the jax reference (rel err 0.0).
"""

import numpy as np
from contextlib import ExitStack

import concourse.bacc as bacc
import concourse.bass as bass
import concourse.mybir as mybir
import concourse.tile as tile
from concourse import bass_utils

B, K, CN, H, W = 4, 16, 256, 64, 64
HW = H * W            # 4096 pixels per batch element
HALF = HW // 2        # 2048 pixels per core
NCORES = 8
NT = HALF // 128      # 16 pixel tiles of 128
NIW = HALF // 16      # 128 wrapped-index slots per candidate

f32 = mybir.dt.float32
i16 = mybir.dt.int16


def build_program():
    nc = bacc.Bacc("TRN2", target_bir_lowering=False, debug=False,
                   num_swdge_queues=4)

    refT_d = nc.dram_tensor("refT", (HW, CN), f32, kind="ExternalInput")
    it_d = nc.dram_tensor("it", (128, NT * CN), f32, kind="ExternalInput")
    wi_d = nc.dram_tensor("wi", (128, K * NIW), i16, kind="ExternalInput")
    xt_d = nc.dram_tensor("xt", (128, NT * K), f32, kind="ExternalInput")
    yt_d = nc.dram_tensor("yt", (128, NT * K), f32, kind="ExternalInput")
    revc_d = nc.dram_tensor("revc", (128, NT * K), f32, kind="ExternalInput")
    ox_d = nc.dram_tensor("ox", (128, NT), f32, kind="ExternalOutput")
    oy_d = nc.dram_tensor("oy", (128, NT), f32, kind="ExternalOutput")

    with ExitStack() as ctx:
        tc = ctx.enter_context(tile.TileContext(nc))
        pers = ctx.enter_context(tc.tile_pool(name="pers", bufs=1))
        gpool = ctx.enter_context(tc.tile_pool(name="g", bufs=6))

        # ---- persistent tiles -------------------------------------------------
        it = pers.tile([128, NT * CN], f32, tag="it")    # inp, pixel-major
        wi = pers.tile([128, K * NIW], i16, tag="wi")
        xt = pers.tile([128, NT * K], f32, tag="xt")     # x, pixel-major
        yt = pers.tile([128, NT * K], f32, tag="yt")
        revc = pers.tile([128, NT * K], f32, tag="revc")
        st = pers.tile([128, NT * K], f32, tag="st")     # S, pixel-major

        st_g = st[:].rearrange("p (t j) -> p t j", j=K)

        # wi first (descriptor generation reads it), it second (the first
        # multiply needs it); xt/yt/revc ride behind the first gathers.
        nc.sync.dma_start(wi[:], wi_d.ap())
        nc.sync.dma_start(it[:], it_d.ap())

        # 512-index chunks: the 1024-descriptor SWDGE ring then holds two
        # chunks per queue, so descriptor generation for the next chunk never
        # stalls on the previous chunk's drain; rotate across all 4 queues.
        CH = 512
        NCH = HALF // CH          # 4 chunks per candidate
        for k in range(K):
            g = gpool.tile([128, NT * CN], f32, tag="g", name=f"g{k}")
            for h2 in range(NCH):
                nc.gpsimd.dma_gather(
                    g[:, NT * CN // NCH * h2:NT * CN // NCH * (h2 + 1)]
                        .rearrange("p (j e) -> p j e", e=CN),
                    refT_d[:],
                    wi[:, k * NIW + (CH // 16) * h2:
                        k * NIW + (CH // 16) * (h2 + 1)],
                    CH, CH, CN,
                    queue_num=(k * NCH + h2) % 4,
                )
            if k == 0:
                nc.sync.dma_start(xt[:], xt_d.ap())
                nc.sync.dma_start(yt[:], yt_d.ap())
                nc.sync.dma_start(revc[:], revc_d.ap())
            # Whole-candidate DVE ops (fewer, larger instructions)
            nc.vector.tensor_mul(g[:], g[:], it[:])
            nc.vector.tensor_reduce(
                st_g[:, :, k],
                g[:].rearrange("p (j e) -> p j e", e=CN),
                axis=mybir.AxisListType.X, op=mybir.AluOpType.add,
            )

        # ---- argmax (first occurrence) + offset select ------------------------
        def grp(ap):  # (128, 256) -> (128, 16, 16)
            return ap.rearrange("p (t j) -> p t j", j=K)

        gmax = pers.tile([128, NT], f32, tag="gmax")
        t1 = pers.tile([128, NT * K], f32, tag="t1")
        r1 = pers.tile([128, NT], f32, tag="r1")
        oh1 = pers.tile([128, NT * K], f32, tag="oh1")
        sel = pers.tile([128, NT * K], f32, tag="sel")
        oxv = pers.tile([128, NT], f32, tag="oxv")
        oyv = pers.tile([128, NT], f32, tag="oyv")

        nc.vector.tensor_reduce(gmax[:], st_g, axis=mybir.AxisListType.X,
                                op=mybir.AluOpType.max)
        gb = gmax[:].unsqueeze(2).broadcast_to((128, NT, K))
        nc.vector.tensor_tensor(grp(t1[:]), st_g, gb,
                                op=mybir.AluOpType.is_equal)
        nc.vector.tensor_mul(t1[:], t1[:], revc[:])
        nc.vector.tensor_reduce(r1[:], grp(t1[:]), axis=mybir.AxisListType.X,
                                op=mybir.AluOpType.max)
        rb = r1[:].unsqueeze(2).broadcast_to((128, NT, K))
        nc.vector.tensor_tensor(grp(oh1[:]), grp(t1[:]), rb,
                                op=mybir.AluOpType.is_equal)
        nc.vector.tensor_mul(sel[:], oh1[:], xt[:])
        nc.vector.tensor_reduce(oxv[:], grp(sel[:]), axis=mybir.AxisListType.X,
                                op=mybir.AluOpType.add)
        nc.vector.tensor_mul(sel[:], oh1[:], yt[:])
        nc.vector.tensor_reduce(oyv[:], grp(sel[:]), axis=mybir.AxisListType.X,
                                op=mybir.AluOpType.add)

        nc.sync.dma_start(ox_d.ap(), oxv[:])
        nc.sync.dma_start(oy_d.ap(), oyv[:])

    nc.compile()
    return nc


def make_consts():
    # revc[p, t*16 + j] = 16 - j : reverse weight for first-occurrence argmax
    return np.tile(
        np.tile((16.0 - np.arange(16, dtype=np.float32)), NT).reshape(1, NT * K),
        (128, 1),
    ).copy()


def make_in_maps(input_features, ref_features, aggregated_x, aggregated_y):
    revc = make_consts()
    in_maps = []
    for core in range(NCORES):
        b, h = core // 2, core % 2
        sl = slice(h * HALF, (h + 1) * HALF)
        inp = np.asarray(input_features[b]).reshape(CN, HW)[:, sl]
        xh = np.asarray(aggregated_x[b]).reshape(K, HW)[:, sl]
        yh = np.asarray(aggregated_y[b]).reshape(K, HW)[:, sl]
        idx = (xh + 64.0 * yh).astype(np.int16)              # (K, HALF)
        # SWDGE wrapped index layout: list elem i at partition i%16, slot
        # i//16, replicated across the 8 16-partition blocks.
        wi = np.tile(
            idx.reshape(K, NIW, 16).transpose(2, 0, 1).reshape(16, -1),
            (8, 1),
        )
        in_maps.append({
            "refT": np.ascontiguousarray(
                np.asarray(ref_features[b]).reshape(CN, HW).T),
            "it": np.ascontiguousarray(
                inp.reshape(CN, NT, 128).transpose(2, 1, 0).reshape(128, -1)),
            "wi": np.ascontiguousarray(wi),
            "xt": np.ascontiguousarray(
                xh.reshape(K, NT, 128).transpose(2, 1, 0).reshape(128, -1)),
            "yt": np.ascontiguousarray(
                yh.reshape(K, NT, 128).transpose(2, 1, 0).reshape(128, -1)),
            "revc": revc,
        })
    return in_maps


def assemble_outputs(results):
    offset_x = np.empty((B, 1, H, W), dtype=np.float32)
    offset_y = np.empty((B, 1, H, W), dtype=np.float32)
    for core in range(NCORES):
        b, h = core // 2, core % 2
        sl = slice(h * HALF, (h + 1) * HALF)
        # ox[p, t] holds pixel t*128+p -> transpose to pixel order
        offset_x[b, 0].reshape(HW)[sl] = results[core]["ox"].T.reshape(HALF)
        offset_y[b, 0].reshape(HW)[sl] = results[core]["oy"].T.reshape(HALF)
    return offset_x, offset_y


_PROGRAM = None


def kernel(input_features, ref_features, aggregated_x, aggregated_y):
    global _PROGRAM
    if _PROGRAM is None:
        _PROGRAM = build_program()
    nc = _PROGRAM
    in_maps = make_in_maps(input_features, ref_features, aggregated_x, aggregated_y)
    res = bass_utils.run_bass_kernel_spmd(nc, in_maps, core_ids=list(range(NCORES)))
    return assemble_outputs(res.results)
